# revision 32
# baseline (speedup 1.0000x reference)
"""NextVLAD Trainium2 kernel — 8-way data-parallel over batch (1 sample/core).

v10 dataflow per core (M=512 tokens, N=1024, E*N=2048, G=8, K=128, D=256):
  host packs every tensor in final SBUF partition layout; W12 = W_inp.T @
  W_gk.T is precomputed on host so BOTH the logits and the VLAD operand are
  produced directly in [m, *] orientation with x-slices as the stationary
  matmul operand (contraction over n=1024, fp8 DoubleRow). No on-chip
  transposes of y, no [e, m] intermediate at all.

  ss    = ones-DR-matmul over xsq ((x/4)^2 fp8, Scalar Square)
  nrm   = sqrt(ss*4) = 4||x|| (f32r); invP (per-token [128,4]) via 4 PE
          transposes of the row-identical nrm + recip_approx_fast
  sg    = sigmoid(xT W1g * invP/16 + bg)        [m, 8] direct
  yTraw = xT W1 accumulated per (m, e-slice)    [m, e] psum waves
  yT    = yTraw*invP2 + 32*b_inp -> bf16 (g,h)-blocks, ones col = 32
  lg    = xT W12; ex = exp(lg * invPe)  f32r    (b-terms cancel in softmax)
  se    = ones-matmul ; ise = recip_approx_fast ; wf = (ex*sgc)*ise bf16
  vd[k, 258] accumulates 8g x 4m ; vlad = vd - S*cent
  out   = vlad * rsqrt(sumsq*128)   (global l2 norm == /sqrt(128))
"""
import os
import numpy as np

N = 1024          # feature size
EN = 2048         # expanded features
G = 8             # groups
KC = 128          # clusters
D = 256           # per-group cluster dim
BW = D + 2        # group block width in yT (data + ones + pad)
M = 512           # tokens per sample (8*8*8)
ET = EN // 128    # 16 e-tiles
MT = 4            # m-tiles of 128
W1W = EN + G + 8  # padded w1 block (step%16==0 for DoubleRow)
W12S = 256.0      # host scale on W12

_cache = {}


def _build_nc():
    import concourse.bacc as bacc
    import concourse.tile as tile
    from concourse import mybir

    f32 = mybir.dt.float32
    f32r = mybir.dt.float32r
    bf16 = mybir.dt.bfloat16
    fp8 = mybir.dt.float8e4
    Alu = mybir.AluOpType
    Act = mybir.ActivationFunctionType
    DR = mybir.MatmulPerfMode.DoubleRow

    nc = bacc.Bacc("TRN2", target_bir_lowering=False)
    xt_d = nc.dram_tensor("xt", [128, 4 * 1024], fp8, kind="ExternalInput")
    w1_d = nc.dram_tensor("w1", [128, 4 * 2 * W1W], fp8, kind="ExternalInput")
    w12_d = nc.dram_tensor("w12", [128, 4 * 2 * 1024], fp8, kind="ExternalInput")
    cf_d = nc.dram_tensor("cf", [128, D + G], f32, kind="ExternalInput")   # -cent|bgb
    crr_d = nc.dram_tensor("crr", [128, 256], f32r, kind="ExternalInput")  # ones|eye128
    binp_d = nc.dram_tensor("binp", [1, EN], f32, kind="ExternalInput")    # 32*b_inp
    out_d = nc.dram_tensor("out", [KC, D], f32, kind="ExternalOutput")

    with tile.TileContext(nc) as tc:
        with tc.tile_pool(name="const", bufs=1) as constp, \
             tc.tile_pool(name="work", bufs=1) as work, \
             tc.tile_pool(name="dram", bufs=1, space="DRAM") as dramp, \
             tc.tile_pool(name="ps", bufs=1, space="PSUM") as ps:
            # ---------------- input DMAs (dense, pre-packed) ----------------
            # separate SBUF tile per chunk: tile-granular deps unlock compute
            # as soon as each chunk's DMA lands
            xc = [constp.tile([128, 2048], fp8, name=f"xc{i}") for i in range(2)]
            w1c = [constp.tile([128, 2 * W1W], fp8, name=f"w1c{c}") for c in range(4)]
            w12c = [constp.tile([128, 2048], fp8, name=f"w12c{c}") for c in range(4)]
            cf = constp.tile([128, D + G], f32, name="cf")
            crr = constp.tile([128, 256], f32r, name="crr")
            binp_b = constp.tile([128, EN], f32, name="binp")
            # x + w1 first (sync/scalar queues); w12/binp deferred to gpsimd
            # so they don't steal HBM bandwidth from the critical-path loads
            # two balanced queue chains for x+w1 (arrival order: c0,c2,c1,c3);
            # w12/binp gated behind w1 so the critical loads get full bandwidth
            nc.sync.dma_start(out=xc[0][:], in_=xt_d[:, 0:2048])
            nc.scalar.dma_start(out=xc[1][:], in_=xt_d[:, 2048:4096])
            for c, eng in ((0, nc.sync), (2, nc.scalar), (1, nc.sync), (3, nc.scalar)):
                eng.dma_start(out=w1c[c][:],
                              in_=w1_d[:, c * 2 * W1W:(c + 1) * 2 * W1W])
            nc.gpsimd.dma_start(out=crr[:], in_=crr_d[:])
            nc.gpsimd.dma_start(out=cf[:], in_=cf_d[:])
            w2gate = dramp.tile([1, 64], fp8, name="w2gate")
            nc.gpsimd.dma_start(out=w2gate[:], in_=w1c[3][0:1, 0:64])
            nc.gpsimd.dma_start(out=binp_b[:], in_=binp_d[:].to_broadcast([128, EN]))
            for c in range(4):
                nc.gpsimd.dma_start(out=w12c[c][:],
                                    in_=w12_d[:, c * 2048:(c + 1) * 2048])
            centn_t = cf[:, 0:D]
            bgb_t = cf[:, D:D + G]
            ones_r = crr[:, 0:128]
            identr_t = crr[:, 128:256]

            xv = [t.rearrange("p (c s m) -> p c s m", c=2, m=M) for t in xc]
            w1v = [t.rearrange("p (s e) -> p s e", e=W1W) for t in w1c]
            w12v = [t.rearrange("p (s j) -> p s j", j=1024) for t in w12c]

            def xch(c):
                return xv[c // 2][:, c % 2]

            def xst(c, m):
                # x chunk c, m-tile block as DoubleRow stationary [p, 2, 128]
                return xv[c // 2][:, c % 2, :, m * 128:(m + 1) * 128]

            dum = work.tile([1, 1], f32, name="dum")
            nc.vector.memset(dum[:], 1.0)

            crb = work.tile([128, 128], bf16, name="crb")
            nc.vector.memset(crb[:], 1.0)
            yT_t = [work.tile([128, G * BW], bf16, name=f"yT{m}") for m in range(MT)]
            for m in range(MT):
                # ones col (=32, matches global 32 scale) and zero pad col
                nc.gpsimd.memset(yT_t[m].rearrange("p (g c) -> p g c", c=BW)[:, :, D:D + 1], 32.0)
                nc.gpsimd.memset(yT_t[m].rearrange("p (g c) -> p g c", c=BW)[:, :, D + 1:D + 2], 0.0)

            nrmP = work.tile([128, MT], f32, name="nrmP")
            invP = work.tile([128, MT], f32, name="invP")     # = 0.25/||x||
            invP2 = work.tile([128, MT], f32, name="invP2")   # = 0.5/||x||
            invPe = work.tile([128, MT], f32, name="invPe")   # exp scale
            invPg = work.tile([128, MT], f32, name="invPg")   # gate scale

            # --------- sigmoid gate matmuls first (tiny, x+w1c deps only) ----
            sgr_t = [work.tile([128, G], f32, name=f"sgr{m}") for m in range(MT)]
            for m in range(MT):
                sg_ps = ps.tile([128, G], f32, name="sg_ps", tag="tp", bufs=4)
                for i, c in enumerate((0, 2, 1, 3)):
                    nc.tensor.matmul(sg_ps[:], xst(c, m), w1v[c][:, :, EN:EN + G],
                                     start=(i == 0), stop=(i == 3), perf_mode=DR)
                nc.vector.tensor_copy(sgr_t[m][:], sg_ps[:])  # free psum early

            def y_mms(es, tag):
                yw_ps = [ps.tile([128, 512], f32, name=f"yw{es}_{m}", tag=tag,
                                 bufs=4) for m in range(MT)]
                for i, c in enumerate((0, 2, 1, 3)):
                    for m in range(MT):
                        nc.tensor.matmul(yw_ps[m][:], xst(c, m),
                                         w1v[c][:, :, es * 512:(es + 1) * 512],
                                         start=(i == 0), stop=(i == 3),
                                         perf_mode=DR)
                return yw_ps

            def y_stt(es, yw_ps):
                for m in range(MT):
                    nc.vector.scalar_tensor_tensor(
                        out=yT_t[m].rearrange("p (g c) -> p g c", c=BW)
                                   [:, 2 * es:2 * es + 2, 0:D]
                                   .rearrange("p g (h j) -> p g h j", j=128),
                        in0=yw_ps[m].rearrange("p (g h j) -> p g h j", g=2, j=128),
                        scalar=invP2[:, m:m + 1],
                        in1=binp_b[:, es * 512:(es + 1) * 512]
                               .rearrange("p (g h j) -> p g h j", g=2, j=128),
                        op0=Alu.mult, op1=Alu.add)

            yw0 = y_mms(0, "ps")

            # ---------------- ss -> invP family (overlaps wave 0) ----------
            # xsq split across DVE and Scalar so neither serializes startup
            xsqA = work.tile([128, 2048], bf16, name="xsqA")
            xsqB = work.tile([128, 2048], bf16, name="xsqB")
            for c in range(2):
                xf = xc[0][:, c * 1024:(c + 1) * 1024]
                nc.vector.tensor_mul(xsqA[:, c * 1024:(c + 1) * 1024], xf, xf)
                xg = xc[1][:, c * 1024:(c + 1) * 1024]
                nc.scalar.activation(xsqB[:, c * 1024:(c + 1) * 1024], xg,
                                     Act.Square)
            xsqv = [t.rearrange("p (c s m) -> p c s m", c=2, m=M)
                    for t in (xsqA, xsqB)]
            ss_ps = ps.tile([128, M], f32, name="ss_ps", tag="tp", bufs=4)
            for i, c in enumerate((0, 2, 1, 3)):
                for s in range(2):
                    nc.tensor.matmul(ss_ps[:], crb[:], xsqv[c // 2][:, c % 2, s],
                                     start=(i == 0 and s == 0),
                                     stop=(i == 3 and s == 1))
            nrm_t = work.tile([128, M], f32r, name="nrm")
            nc.scalar.activation(nrm_t[:], ss_ps[:], Act.Sqrt, scale=0.25)  # 4||x||
            # per-token 1/||x|| family in [m, *] layout via PE transpose of the
            # row-identical nrm tile (column 0 of each block = per-token nrm)
            nrt_ps = ps.tile([128, 512], f32, name="nrt_ps", tag="tp", bufs=4)
            for t in range(MT):
                nc.tensor.transpose(nrt_ps[:, t * 128:(t + 1) * 128].bitcast(f32r),
                                    nrm_t[:, t * 128:(t + 1) * 128],
                                    identr_t)
            nc.vector.tensor_copy(nrmP[:],
                                  nrt_ps.rearrange("p (t j) -> p t j", j=128)[:, :, 0])
            nc.vector.reciprocal_approx_fast(out=invP[:], in_=nrmP[:])
            nc.vector.tensor_scalar_mul(invP2[:], invP[:], 2.0)
            nc.vector.tensor_scalar_mul(invPe[:], invP[:], 0.5 / W12S)
            nc.vector.tensor_scalar_mul(invPg[:], invP[:], 1.0 / 16.0)

            # sigmoid gates from the raw copies
            sgc_t = [work.tile([128, G], f32, name=f"sgc{m}") for m in range(MT)]
            for m in range(MT):
                nc.vector.scalar_tensor_tensor(out=sgc_t[m][:], in0=sgr_t[m][:],
                                               scalar=invPg[:, m:m + 1], in1=bgb_t,
                                               op0=Alu.mult, op1=Alu.add)
                nc.scalar.activation(sgc_t[m][:], sgc_t[m][:], Act.Sigmoid)

            # remaining yT waves (alternating PSUM tags)
            y_stt(0, yw0)
            for es, tag in ((1, "tp"), (2, "ps"), (3, "tp")):
                y_stt(es, y_mms(es, tag))

            # ---------------- phase 2: logits via W12, exp ----------------
            ex_t = [[work.tile([128, 512], f32r, name=f"ex{m}_{h}") for h in range(2)]
                    for m in range(MT)]
            lg_ps = [[ps.tile([128, 512], f32, name=f"lg{m}_{h}",
                               tag=("ps" if h == 0 else "tp"), bufs=4)
                      for h in range(2)] for m in range(MT)]
            for h in range(2):
                for i, c in enumerate((0, 2, 1, 3)):
                    for m in range(MT):
                        nc.tensor.matmul(lg_ps[m][h][:], xst(c, m),
                                         w12v[c][:, :, h * 512:(h + 1) * 512],
                                         start=(i == 0), stop=(i == 3),
                                         perf_mode=DR)
                for m in range(MT):
                    nc.scalar.activation(ex_t[m][h][:], lg_ps[m][h][:],
                                         Act.Exp, scale=invPe[:, m:m + 1])
            # final-Sqrt table preload: depends on last ex tile so the
            # scheduler cannot hoist it before the exps
            nc.scalar.activation(dum[:], ex_t[3][1][0:1, 0:1].bitcast(f32), Act.Sqrt)

            # ---------------- phase 3: softmax denom, weights, einsum --------
            ise_t = [work.tile([128, 512], f32, name=f"ise{h}") for h in range(2)]
            wf_t = [[work.tile([128, KC], bf16, name=f"wf{m}_{g}") for g in range(G)]
                    for m in range(MT)]
            vd_ps = ps.tile([128, 512], f32, name="vd_ps", tag="ps", bufs=4)[:, 0:BW]

            k = 0
            for h in range(2):
                se_ps = ps.tile([128, 512], f32, name=f"se{h}",
                                tag=("ps" if h == 0 else "tp"), bufs=4)
                for m in range(MT):
                    nc.tensor.matmul(se_ps[:], ones_r, ex_t[m][h][:],
                                     start=(m == 0), stop=(m == MT - 1))
                nc.vector.reciprocal_approx_fast(out=ise_t[h][:], in_=se_ps[:])
                for g in range(h * 4, h * 4 + 4):
                    lc = g * KC - h * 512
                    for m in range(MT):
                        nc.vector.scalar_tensor_tensor(
                            out=wf_t[m][g][:],
                            in0=ex_t[m][h][:, lc:lc + KC].bitcast(f32),
                            scalar=sgc_t[m][:, g:g + 1], in1=ise_t[h][:, lc:lc + KC],
                            op0=Alu.mult, op1=Alu.mult)
                    for m in range(MT):
                        nc.tensor.matmul(vd_ps[:], wf_t[m][g][:],
                                         yT_t[m][:, g * BW:(g + 1) * BW],
                                         start=(k == 0), stop=(k == G * MT - 1))
                        k += 1

            # ---------------- final: centroid fixup + l2 norm ----------------
            vlad_t = work.tile([128, D], f32, name="vlad")
            nc.vector.scalar_tensor_tensor(
                out=vlad_t[:], in0=centn_t[:], scalar=vd_ps[:, D:D + 1],
                in1=vd_ps[:, 0:D], op0=Alu.mult, op1=Alu.add)
            sq_t = work.tile([128, D], f32, name="sq")
            ss2_t = work.tile([128, 1], f32, name="ss2")
            nc.vector.scalar_tensor_tensor(
                out=sq_t[:], in0=vlad_t[:], scalar=1.0, in1=vlad_t[:],
                op0=Alu.bypass, op1=Alu.mult, accum_out=ss2_t[:])
            nr2_t = work.tile([128, 1], f32, name="nr2")
            nc.scalar.activation(nr2_t[:], ss2_t[:], Act.Sqrt, scale=128.0)
            r1_t = work.tile([128, 1], f32, name="r1")
            nc.vector.reciprocal_approx_fast(out=r1_t[:], in_=nr2_t[:])
            out_t = work.tile([128, D], f32, name="out")
            nc.vector.tensor_scalar_mul(out_t[:], vlad_t[:], r1_t[:])
            nc.sync.dma_start(out=out_d[:], in_=out_t[:])

    nc.compile()
    return nc


def _get_nc():
    if "nc" not in _cache:
        _cache["nc"] = _build_nc()
    return _cache["nc"]


def kernel(x, W_inp, b_inp, W_g, b_g, W_gk, b_gk, centroids):
    from concourse.bass_utils import run_bass_kernel_spmd
    import ml_dtypes as mld

    nc = _get_nc()

    x = np.asarray(x, dtype=np.float32)
    X = x.reshape(8, 8, N, 64).transpose(0, 2, 1, 3).reshape(8, N, M)
    WgT = ((np.asarray(W_g, np.float64) @ np.asarray(W_inp, np.float64)).T
           ).astype(np.float32)
    W1 = np.zeros((N, W1W), np.float32)
    W1[:, 0:EN] = np.asarray(W_inp, np.float32).T
    W1[:, EN:EN + G] = WgT
    W1 = np.ascontiguousarray(
        (W1 * 8.0).reshape(4, 2, 128, W1W).transpose(2, 0, 1, 3)
        .reshape(128, 4 * 2 * W1W).astype(mld.float8_e4m3))
    W12 = (np.asarray(W_inp, np.float64).T @ np.asarray(W_gk, np.float64).T
           ).astype(np.float32)
    W12 = np.ascontiguousarray(
        (W12 * W12S).reshape(4, 2, 128, 1024).transpose(2, 0, 1, 3)
        .reshape(128, 4 * 2048).astype(mld.float8_e4m3))
    bg = (np.asarray(b_g, np.float64)
          + np.asarray(W_g, np.float64) @ np.asarray(b_inp, np.float64)
          ).astype(np.float32)
    binp = np.ascontiguousarray(
        np.asarray(b_inp, np.float32).reshape(1, EN) * 32.0)
    cf = np.zeros((128, D + G), np.float32)
    cf[:, 0:D] = -np.asarray(centroids, np.float32)
    cf[:, D:D + G] = bg.reshape(1, G)
    crr = np.concatenate([np.ones((128, 128), np.float32),
                          np.eye(128, dtype=np.float32)], axis=1)

    in_maps = []
    for b in range(8):
        xb = np.ascontiguousarray(
            (X[b] * 8.0).reshape(4, 2, 128, M).transpose(2, 0, 1, 3)
            .reshape(128, 4096).astype(mld.float8_e4m3))
        in_maps.append({
            "xt": xb, "w1": W1, "w12": W12, "cf": cf, "crr": crr, "binp": binp,
        })

    trace = os.environ.get("KERNEL_TRACE") == "1"
    r = run_bass_kernel_spmd(nc, in_maps, core_ids=list(range(8)), trace=trace)
    _cache["last_results"] = r
    return np.stack([r.results[b]["out"].reshape(KC * D) for b in range(8)]).astype(np.float32)
